# revision 1
# baseline (speedup 1.0000x reference)
"""Binarized 3x3 conv (BConv2d) on 8 TRN2 NeuronCores — 2x2 output-blocked.

Problem: x (32, 32, 256, 256) f32, weight (32, 32, 3, 3) f32.
  out = conv2d(x, sign(weight), padding='same') / sqrt(32*9)

TimelineSim: 117168 ns/core (baseline image-block-diagonal kernel: 254373).
Breakdown: ~3.99us input/weights-latency startup + 109.1us gapless matmul
stream
+ 4.1us drain/DMA/barrier tail.  The stream is a proven floor: each 2x2
output block reads a 4x4x32ci = 512-value input patch through the
128-value/cycle rhs port, so >= 4 passes per block; (2,2) blocking
minimizes passes/(outputs per block) over all by*bx <= 4 blockings.

Strategy (v2):
  - Data-parallel over batch: core i gets images 4i..4i+3 (no collectives).
  - 2x2 output blocking: every matmul computes M = 128 = (yo, xo, co)
    output columns — 2x2 spatial phases x 32 output channels — over
    free positions (y0, x0) (the 2x2-decimated grid).  K = 128 =
    (jx, ci): 4 x-shift phase groups x 32 input channels.  The 3x3
    stencil collapses into 4 PSUM-accumulated matmuls per strip
    (jy = 0..3 row offsets), each with a dense-packed stationary
    W_jy[(jx,ci),(yo,xo,co)] = sign(w)[co,ci,jy-yo,jx-xo].
    PE utilization: 36864 useful MACs per 4*16384 slots = 56.25%
    (vs 25% for the image-block-diagonal formulation).
  - Input layout: host pre-pads each image to 258x258, de-interleaves
    even/odd columns into two 129-wide phase planes (fp16), which DMA
    straight onto partitions 0..63.  Phase groups g2/g3 (x-shift +2,
    +3) are the same planes shifted one phase column; they are produced
    on-chip by a single cross-partition DVE copy per row-chunk
    (partitions 0..63 -> 64..127), off the DMA critical path.
  - PSUM fp32 accumulate; Act engine drains with the 1/sqrt(288) scale
    to fp16; output leaves in (m, y0, x0) plane order and the host
    de-scrambles to NCHW.
  - Queueing: input DMAs on SP, output DMAs on SP after them per image,
    weights on Act, shift-copies on DVE, drains on Act — tuned so no
    stream head-of-line-blocks another (each engine SEQ waits its
    instructions' semaphores inline).
  - Per-core engine busy: PE 111us (bound), DMA 94us, Act ~78us,
    DVE ~35us, HWDGE ~83us.
"""

import numpy as np

import concourse.mybir as mybir
import concourse.tile as tile
from concourse import bacc
from concourse import bass_utils

N_CORES = 8
N_IMG = 4          # images per core
C_IN = 32
C_OUT = 32
K = 3
H = 256
W = 256
DIV = float(np.sqrt(C_IN * K * K))

HP = H + 2         # padded rows
WPH = W // 2 + 1   # phase-plane width (129)
NY0 = H // 2       # decimated rows (128)
NX0 = W // 2       # decimated cols (128)
SC_Y0 = 4          # y0 rows per strip (8 output rows; 512 PSUM slots)
N_STRIPS = NY0 // SC_Y0  # 32 strips per image


def build_conv_kernel(repeats=1, warmup_mms=4, in_chunks=17, strips_per_dma=2,
                      out_eng="sync", ppool_bufs=6, ot_bufs=6, first_chunk=16,
                      drain_split=False, w_eng="sync2", taper=True,
                      tail_mode="pair", custom_bounds=None, fast_start=True,
                      w_slab=False):
    """Build the per-core Bass graph.  Returns nc (compiled Bacc)."""
    nc = bacc.Bacc(
        "TRN2", target_bir_lowering=False, debug=False, num_devices=N_CORES
    )
    # host-marshalled fp16 phase planes: [img, 64=(E:32ci, O:32ci), 258, 129]
    x_dram = nc.dram_tensor(
        "xin", [N_IMG, 64, HP, WPH], mybir.dt.float16, kind="ExternalInput"
    )
    # stationary weights: [K=128, jy=4, M=128]
    w_dram = nc.dram_tensor(
        "w4", [128, 4, 128], mybir.dt.float16, kind="ExternalInput"
    )
    # output planes: [img, m=(yo,xo,co)=128, y0=128, x0=128]
    out_dram = nc.dram_tensor(
        "out", [N_IMG, 128, NY0, NX0], mybir.dt.float16, kind="ExternalOutput"
    )

    # input row-chunk boundaries: two small leading chunks (16, 10 rows)
    # gate the first strips with minimum latency, then a steady 16-row
    # cadence (swept via TimelineSim; uniform grids are ~200ns slower)
    if custom_bounds is not None:
        bounds = list(custom_bounds)
        assert bounds[0] == 0 and bounds[-1] == HP
    elif in_chunks == 17 and first_chunk == 16:
        bounds = [0, 16, 26]
        while bounds[-1] < HP:
            bounds.append(min(bounds[-1] + 16, HP))
    else:
        first = first_chunk
        bounds = [0, first]
        step = (HP - first + in_chunks - 2) // (in_chunks - 1) if in_chunks > 1 else HP
        while bounds[-1] < HP:
            bounds.append(min(bounds[-1] + step, HP))

    with tile.TileContext(nc) as tc:
        with (
            tc.tile_pool(name="persist", bufs=1) as perpool,
            tc.tile_pool(name="img", bufs=2) as imgpool,
            tc.tile_pool(name="ostage", bufs=ot_bufs) as opool,
            tc.tile_pool(name="psum", bufs=ppool_bufs, space="PSUM") as ppool,
            tc.tile_pool(name="pwarm", bufs=1, space="PSUM") as wpool,
        ):
            wsb = perpool.tile([128, 4, 128], mybir.dt.float16, name="wsb")
            wz = perpool.tile([128, 128], mybir.dt.float16, name="wz")
            warm = perpool.tile([128, 512], mybir.dt.float16, name="warm")
            nc.gpsimd.memset(wz[:], 0.0)
            nc.gpsimd.memset(warm[:], 0.0)
            # weights issue SECOND on SP (right after input chunk 0, emitted
            # inside load_image below): same HWDGE slot as via Act, but SP's
            # shorter dge_dma_delay (650 vs 784) lands the weights sem
            # ~140ns earlier — it is the exact gate of the first real matmul
            if w_eng != "sync2":
                getattr(nc, w_eng).dma_start(out=wsb[:], in_=w_dram[:, :, :])

            # PE warm-up: zero-weight matmuls while the first input chunks
            # are in flight keep the PE p-state ramp off the critical path.
            if warmup_mms:
                wpt = wpool.tile([128, 512], mybir.dt.float32, name="wpt")
                for _ in range(warmup_mms):
                    nc.tensor.matmul(
                        wpt[:], wz[:], warm[:], start=True, stop=True
                    )

            def load_image(i):
                # input DMAs on SP; output DMAs stay on their own queue, so
                # input prefetch of image i+1 never queues behind image i's
                # outputs.
                t = imgpool.tile(
                    [128, HP, WPH], mybir.dt.float16, name="t", tag="img"
                )
                for r0, r1 in zip(bounds[:-1], bounds[1:]):
                    nc.sync.dma_start(
                        out=t[0:64, r0:r1, :], in_=x_dram[i, :, r0:r1, :]
                    )
                    if w_eng == "sync2" and i == 0 and r0 == 0:
                        if w_slab:
                            nc.sync.dma_start(
                                out=wsb[:, 0:1, :], in_=w_dram[:, 0:1, :]
                            )
                            nc.sync.dma_start(
                                out=wsb[:, 1:4, :], in_=w_dram[:, 1:4, :]
                            )
                        else:
                            nc.sync.dma_start(out=wsb[:], in_=w_dram[:, :, :])
                copy_ranges = list(zip(bounds[:-1], bounds[1:]))
                if i == 0 and bounds[1] > 10:
                    # progressive first-chunk copies matched to what the
                    # first strips' matmuls need row-by-row
                    if fast_start:
                        head = [(0, 3), (3, 7), (7, 10), (10, bounds[1])]
                    else:
                        head = [(0, 7), (7, 10), (10, bounds[1])]
                    copy_ranges = head + copy_ranges[1:]
                for r0, r1 in copy_ranges:
                    # phase groups g2/g3 = g0/g1 shifted one phase col
                    nc.vector.tensor_copy(
                        t[64:128, r0:r1, 0 : WPH - 1], t[0:64, r0:r1, 1:WPH]
                    )
                return t

            def emit_block(i, y0a, ny0, t, drain_eng):
                """One accumulation group covering y0 rows [y0a, y0a+ny0):
                4 jy-matmuls -> PSUM -> drain -> its own output DMA."""
                pt = ppool.tile(
                    [128, ny0, NX0], mybir.dt.float32, name="pt", tag="pt",
                )
                r0 = 2 * y0a
                for jy in range(4):
                    nc.tensor.matmul(
                        pt[:],
                        wsb[:, jy, :],
                        t[:, r0 + jy : r0 + jy + 2 * ny0 - 1 : 2, 0:NX0],
                        start=(jy == 0),
                        stop=(jy == 3),
                    )
                ot = opool.tile(
                    [128, ny0, NX0], mybir.dt.float16, name="ot", tag="ot",
                )
                drain_eng(ot, pt)
                getattr(nc, out_eng).dma_start(
                    out=out_dram[i, :, y0a : y0a + ny0, :], in_=ot[:]
                )

            def drain_act(ot, pt, sub=None):
                dst = ot if sub is None else ot[:, sub[0] : sub[1], :]
                nc.scalar.activation(
                    dst, pt[:], mybir.ActivationFunctionType.Copy,
                    scale=1.0 / DIV,
                )

            for _rep in range(repeats):
                for i in range(N_IMG):
                    t = load_image(i)
                    last_img = _rep == repeats - 1 and i == N_IMG - 1
                    for s in range(N_STRIPS):
                        if fast_start and _rep == 0 and i == 0 and s == 0:
                            # emit strip 0 as four 1-row sub-strips: the
                            # first matmul then needs only g2/g3 row 0
                            # (a 1-row copy) instead of rows 0..6, so the
                            # stream starts ~300ns earlier; same total
                            # matmul cycles
                            ot = opool.tile(
                                [128, SC_Y0 * strips_per_dma, NX0],
                                mybir.dt.float16, name="ot", tag="ot",
                            )
                            pts = [
                                ppool.tile(
                                    [128, 1, NX0], mybir.dt.float32,
                                    name="pt", tag="pt",
                                )
                                for _ in range(SC_Y0)
                            ]
                            for jy in range(4):
                                for k in range(SC_Y0):
                                    nc.tensor.matmul(
                                        pts[k][:],
                                        wsb[:, jy, :],
                                        t[:, 2 * k + jy : 2 * k + jy + 1,
                                          0:NX0],
                                        start=(jy == 0),
                                        stop=(jy == 3),
                                    )
                            for k in range(SC_Y0):
                                drain_act(ot, pts[k], (k, k + 1))
                            continue
                        # taper: on the very last image, strips 28-30 share a
                        # triple DMA and strip 31 splits into two half-strips
                        # with their own DMAs, shortening the final
                        # drain->DMA chain after the last matmul
                        use_triple = taper and "triple" in tail_mode
                        use_halve = taper and "halve" in tail_mode
                        use_shift = taper and "shift" in tail_mode
                        if use_shift and last_img:
                            # offset pairing: strip 0 alone, pairs (1,2)...
                            # (29,30), strip 31 alone — the pair preceding
                            # the final DMA completes earlier, freeing HWDGE
                            if s == 0:
                                emit_block(i, 0, SC_Y0, t, drain_act)
                                continue
                            if s < N_STRIPS - 1:
                                h = (s - 1) % strips_per_dma
                                pt = ppool.tile(
                                    [128, SC_Y0, NX0], mybir.dt.float32,
                                    name="pt", tag="pt",
                                )
                                for jy in range(4):
                                    nc.tensor.matmul(
                                        pt[:],
                                        wsb[:, jy, :],
                                        t[:, 8 * s + jy : 8 * s + jy + 7 : 2,
                                          0:NX0],
                                        start=(jy == 0),
                                        stop=(jy == 3),
                                    )
                                if h == 0:
                                    ot = opool.tile(
                                        [128, SC_Y0 * strips_per_dma, NX0],
                                        mybir.dt.float16, name="ot", tag="ot",
                                    )
                                drain_act(ot, pt,
                                          (SC_Y0 * h, SC_Y0 * (h + 1)))
                                if h == strips_per_dma - 1:
                                    y0a = SC_Y0 * (s - h)
                                    getattr(nc, out_eng).dma_start(
                                        out=out_dram[
                                            i, :,
                                            y0a : y0a + SC_Y0 * strips_per_dma,
                                            :,
                                        ],
                                        in_=ot[:],
                                    )
                                continue
                            emit_block(i, SC_Y0 * s, SC_Y0, t, drain_act)
                            continue
                        tail_lo = N_STRIPS - (4 if use_triple else 2)
                        if taper and last_img and s >= tail_lo:
                            if s < N_STRIPS - 1:
                                if use_triple and N_STRIPS - 3 <= s:
                                    continue  # covered by the triple below
                                if not use_triple:
                                    # single-strip DMA for strip N-2
                                    emit_block(i, SC_Y0 * s, SC_Y0, t,
                                               drain_act)
                                    continue
                                if use_triple and s == N_STRIPS - 4:
                                    # triple: strips 28,29,30 -> one DMA
                                    ot = opool.tile(
                                        [128, 3 * SC_Y0, NX0],
                                        mybir.dt.float16, name="ot", tag="ot",
                                    )
                                    for k in range(3):
                                        pt = ppool.tile(
                                            [128, SC_Y0, NX0],
                                            mybir.dt.float32,
                                            name="pt", tag="pt",
                                        )
                                        r0 = 8 * (s + k)
                                        for jy in range(4):
                                            nc.tensor.matmul(
                                                pt[:],
                                                wsb[:, jy, :],
                                                t[:, r0 + jy : r0 + jy + 7 : 2,
                                                  0:NX0],
                                                start=(jy == 0),
                                                stop=(jy == 3),
                                            )
                                        drain_act(
                                            ot, pt,
                                            (SC_Y0 * k, SC_Y0 * (k + 1)),
                                        )
                                    getattr(nc, out_eng).dma_start(
                                        out=out_dram[
                                            i, :,
                                            SC_Y0 * s : SC_Y0 * (s + 3), :
                                        ],
                                        in_=ot[:],
                                    )
                                continue
                            # s == N_STRIPS-1: one strip, one DMA, but the
                            # drain runs split across Act and DVE in
                            # parallel so the final DMA issues sooner
                            pt = ppool.tile(
                                [128, SC_Y0, NX0], mybir.dt.float32,
                                name="pt", tag="pt",
                            )
                            r0 = 8 * s
                            for jy in range(4):
                                nc.tensor.matmul(
                                    pt[:],
                                    wsb[:, jy, :],
                                    t[:, r0 + jy : r0 + jy + 7 : 2, 0:NX0],
                                    start=(jy == 0),
                                    stop=(jy == 3),
                                )
                            ot = opool.tile(
                                [128, SC_Y0, NX0], mybir.dt.float16,
                                name="ot", tag="ot",
                            )
                            if drain_split:
                                nc.scalar.activation(
                                    ot[:, 0:2, :], pt[:, 0:2, :],
                                    mybir.ActivationFunctionType.Copy,
                                    scale=1.0 / DIV,
                                )
                                nc.vector.tensor_scalar_mul(
                                    ot[:, 2:4, :], pt[:, 2:4, :], 1.0 / DIV
                                )
                            else:
                                drain_act(ot, pt)
                            getattr(nc, out_eng).dma_start(
                                out=out_dram[i, :, SC_Y0 * s : SC_Y0 * (s + 1), :],
                                in_=ot[:],
                            )
                            continue
                        pt = ppool.tile(
                            [128, SC_Y0, NX0], mybir.dt.float32,
                            name="pt", tag="pt",
                        )
                        for jy in range(4):
                            nc.tensor.matmul(
                                pt[:],
                                wsb[:, jy, :],
                                t[:, 8 * s + jy : 8 * s + jy + 7 : 2, 0:NX0],
                                start=(jy == 0),
                                stop=(jy == 3),
                            )
                        h = s % strips_per_dma
                        if h == 0:
                            ot = opool.tile(
                                [128, SC_Y0 * strips_per_dma, NX0],
                                mybir.dt.float16, name="ot", tag="ot",
                            )
                        drain_act(ot, pt, (SC_Y0 * h, SC_Y0 * (h + 1)))
                        if h == strips_per_dma - 1:
                            y0a = SC_Y0 * (s - h)
                            getattr(nc, out_eng).dma_start(
                                out=out_dram[
                                    i, :,
                                    y0a : y0a + SC_Y0 * strips_per_dma, :
                                ],
                                in_=ot[:],
                            )

    nc.compile()
    return nc


def make_weight_tensor(weight):
    """[cout,cin,3,3] f32 -> stationary [K=128,(jx,ci)][jy=4][M=128,(yo,xo,co)] fp16."""
    wbin = np.where(weight > 0, 1.0, -1.0).astype(np.float32)
    w4 = np.zeros((128, 4, 128), dtype=np.float16)
    for jy in range(4):
        for jx in range(4):
            for yo in range(2):
                dy = jy - yo
                if not (0 <= dy <= 2):
                    continue
                for xo in range(2):
                    dx = jx - xo
                    if not (0 <= dx <= 2):
                        continue
                    # K = jx*32 + ci ; M = yo*64 + xo*32 + co
                    w4[jx * 32 : jx * 32 + 32, jy,
                       yo * 64 + xo * 32 : yo * 64 + xo * 32 + 32] = \
                        wbin[:, :, dy, dx].T
    return w4


def make_input_planes(x16):
    """x16 (B, 32, 256, 256) fp16 -> (B, 64, 258, 129) padded phase planes."""
    b = x16.shape[0]
    xp = np.zeros((b, C_IN, HP, W + 2), dtype=np.float16)
    xp[:, :, 1 : H + 1, 1 : W + 1] = x16
    ev = xp[:, :, :, 0::2]                      # (b, 32, 258, 129)
    od = xp[:, :, :, 1::2]                      # (b, 32, 258, 129)
    return np.ascontiguousarray(np.concatenate([ev, od], axis=1))


def unscramble_output(arr):
    """(img, 128=(yo,xo,co), 128, 128) fp16 -> (img, 32, 256, 256) f32."""
    a = arr.reshape(-1, 2, 2, C_OUT, NY0, NX0).astype(np.float32)
    # (img, yo, xo, co, y0, x0) -> (img, co, y0, yo, x0, xo)
    a = a.transpose(0, 3, 4, 1, 5, 2)
    return a.reshape(-1, C_OUT, H, W)


def kernel(x, weight, trace=False, repeats=1, _nc_cache={}):
    """Full-input entry point: x (32,32,256,256) f32, weight (32,32,3,3) f32."""
    x = np.asarray(x, dtype=np.float32)
    x16 = x.astype(np.float16)
    weight = np.asarray(weight, dtype=np.float32)
    n_batch = x.shape[0]
    per_core = n_batch // N_CORES

    if repeats not in _nc_cache:
        _nc_cache[repeats] = build_conv_kernel(repeats=repeats)
    nc = _nc_cache[repeats]

    w4 = make_weight_tensor(weight)
    planes = make_input_planes(x16.reshape(-1, C_IN, H, W))
    planes = planes.reshape(N_CORES, per_core, 64, HP, WPH)
    in_maps = [
        {"xin": planes[c], "w4": w4}
        for c in range(N_CORES)
    ]
    try:
        res = bass_utils.run_bass_kernel_spmd(
            nc, in_maps, core_ids=list(range(N_CORES)), trace=trace
        )
    except ModuleNotFoundError:
        res = bass_utils.run_bass_kernel_spmd(
            nc, in_maps, core_ids=list(range(N_CORES)), trace=False
        )
    out = np.concatenate(
        [unscramble_output(r["out"]) for r in res.results], axis=0
    )
    if trace:
        kernel.last_results = res
    return out

